# revision 12
# baseline (speedup 1.0000x reference)
"""Trainium2 Bass kernel for the 6-layer linear-attention MLP block.

Math per layer (reference):
    x  = relu(v @ Dx)                 # [S, N]
    kv = x.T @ v   (per batch)        # [N, D]   contraction over sequence
    a  = x @ kv                       # [S, D]
    y  = relu(a @ Dy) * x             # [S, N]
    v  = ln(v + ln(y @ E))            # [S, D]
final: out = v @ readout              # [S, V]

Sharding (per 4-core batch group, cores 0-3: batch 0, 4-7: batch 1):
row-shard v/a/y/z (512 rows/core) AND column-shard the kv computation
(1024 N-cols/core).  Each core computes two x pieces:
  xT_mine = x[my rows, all N]^T      (needs only my v rows)
  x_nat   = x[all rows, my N-cols]   (needs v of all rows <- AllGather(v))
kv[my cols, :] = x_nat^T @ v_full is then LOCAL in the contraction over
the full sequence -- no AllReduce.  kv chunks are AllGathered (cheap,
~87 GB/s vs AllReduce's ~32 GB/s on this fabric) in 4 pipelined
sub-AGs that overlap the a-phase, which consumes chunks in arrival
order into 8 PSUM accumulation banks.  y/z/LN are row-local; the only
other collective is AllGather(v') at the end of each layer, which rides
under the next layer's xT_mine pass (row-local, so it does not wait).

Everything computes in bf16 with f32 PSUM accumulation; the residual
stream v stays f32.  LayerNorm runs bank-split directly on the z PSUM
banks.  vT comes from HWDGE DMA-transposes of the AllGather output.
"""

import numpy as np
import ml_dtypes

B, S, N, D, V = 2, 2048, 4096, 1024, 32000
L = 6
EPS = 1e-5
NCORES = 8
R = 512    # rows per core
RT = 4     # row tiles of 128
DU = 8     # d tiles of 128
NT = 32    # global n tiles of 128
NL = 8     # local n tiles of 128 (my 1024-col slice)
ST = 16    # s tiles of 128 over the full 2048-row batch
VW = 500
VB = 64
RG = [[0, 1, 2, 3], [4, 5, 6, 7]]

_CACHE = {}


def _build(layers=L):
    import concourse.bacc as bacc
    import concourse.tile as tile
    import concourse.mybir as mybir
    from concourse.masks import make_identity

    f32 = mybir.dt.float32
    bf16 = mybir.dt.bfloat16
    AX = mybir.AxisListType.X
    AF = mybir.ActivationFunctionType
    OP = mybir.AluOpType

    nc = bacc.Bacc("TRN2", target_bir_lowering=False, num_devices=NCORES)

    vmy0 = nc.dram_tensor("vmy0", [RT, 128, D], f32, kind="ExternalInput")
    v0tm = nc.dram_tensor("v0tm", [128, DU, R], bf16, kind="ExternalInput")
    v0f = nc.dram_tensor("v0f", [S, D], bf16, kind="ExternalInput")
    dxs = nc.dram_tensor("dxs", [NT, 128, DU, 128], bf16, kind="ExternalInput")
    dxb = nc.dram_tensor("dxb", [128, DU, 1024], bf16, kind="ExternalInput")
    dyp = nc.dram_tensor("dyp", [NT, 128, DU, 128], bf16, kind="ExternalInput")
    ep = nc.dram_tensor("ep", [NT, 128, D], bf16, kind="ExternalInput")
    rp = nc.dram_tensor("rp", [VB, 128, DU, VW], bf16, kind="ExternalInput")
    out = nc.dram_tensor("out", [RT, 128, V], f32, kind="ExternalOutput")

    with tile.TileContext(nc) as tc:
        with (
            tc.tile_pool(name="constp", bufs=1) as constp,
            tc.tile_pool(name="pers", bufs=1) as pers,
            tc.tile_pool(name="vtfp", bufs=1) as vtfp,
            tc.tile_pool(name="xnp", bufs=1) as xnp,
            tc.tile_pool(name="xtmp", bufs=1) as xtmp,
            tc.tile_pool(name="atp", bufs=1) as atp,
            tc.tile_pool(name="dxsp", bufs=2) as dxsp,
            tc.tile_pool(name="dybp", bufs=2) as dybp,
            tc.tile_pool(name="ecp", bufs=2) as ecp,
            tc.tile_pool(name="kcp", bufs=2) as kcp,
            tc.tile_pool(name="vnp", bufs=2) as vnp,
            tc.tile_pool(name="kvtp", bufs=2) as kvtp,
            tc.tile_pool(name="rbp", bufs=2) as rbp,
            tc.tile_pool(name="opool", bufs=2) as opool,
            tc.tile_pool(name="lnpool", bufs=2) as lnpool,
            tc.tile_pool(name="smpool", bufs=12) as smpool,
            tc.tile_pool(name="psum", bufs=8, space="PSUM") as psum,
            tc.tile_pool(name="dpool", bufs=1, space="DRAM") as dpool,
        ):
            epsc = constp.tile([128, 1], f32)
            nc.vector.memset(epsc[:], EPS)
            ident = constp.tile([128, 128], bf16)
            make_identity(nc, ident)

            v_my = [pers.tile([128, D], f32, name=f"vmy{i}") for i in range(RT)]
            v_bf = [pers.tile([128, D], bf16, name=f"vbf{i}") for i in range(RT)]
            vtm = pers.tile([128, DU, R], bf16, name="vtm")
            dxbt = pers.tile([128, DU, 1024], bf16, name="dxbt")

            vtf = [vtfp.tile([128, S], bf16, name=f"vtf{i}") for i in range(DU)]
            xnat = [xnp.tile([128, 1024], bf16, name=f"xn{i}") for i in range(ST)]
            xtm = [xtmp.tile([128, R], bf16, name=f"xtm{i}") for i in range(NT)]
            at = [atp.tile([128, R], bf16, name=f"at{i}") for i in range(DU)]

            kv_in = [dpool.tile([2, 128, D], bf16, name=f"kvin{q}") for q in range(4)]
            kv_out = [
                dpool.tile([4, 2, 128, D], bf16, name=f"kvout{q}") for q in range(4)
            ]
            v_ag_in = dpool.tile([RT, 128, D], bf16)
            v_ag_out = dpool.tile([S, D], bf16)

            # tiny warmup collective: absorbs the first-op trigger latency
            warm_in = dpool.tile([128, 4], f32)
            warm_out = dpool.tile([128, 4], f32)
            wt = constp.tile([128, 4], f32, name="wt")
            nc.vector.memset(wt[:], 0.0)
            nc.gpsimd.dma_start(warm_in[:], wt[:])
            nc.gpsimd.collective_compute(
                "AllReduce",
                OP.add,
                replica_groups=RG,
                ins=[warm_in[:].opt()],
                outs=[warm_out[:].opt()],
            )

            nc.sync.dma_start(dxbt[:], dxb[:])
            nc.scalar.dma_start(vtm[:], v0tm[:])
            for rc in range(RT):
                nc.gpsimd.dma_start(v_my[rc][:], vmy0[rc])

            def layer_norm(dst, src):
                rs = smpool.tile([128, 1], f32, tag="sm", name="rs")
                nc.vector.reduce_sum(rs[:], src, axis=AX)
                nm = smpool.tile([128, 1], f32, tag="sm", name="nm")
                nc.vector.tensor_scalar_mul(nm[:], rs[:], -1.0 / D)
                sq = lnpool.tile([128, D], f32, tag="ln", name="sq")
                ssq = smpool.tile([128, 1], f32, tag="sm", name="ssq")
                nc.scalar.activation(
                    sq[:], src, AF.Square, bias=nm[:], scale=1.0, accum_out=ssq[:]
                )
                std = smpool.tile([128, 1], f32, tag="sm", name="std")
                nc.scalar.activation(
                    std[:], ssq[:], AF.Sqrt, bias=epsc[:], scale=1.0 / D
                )
                rstd = smpool.tile([128, 1], f32, tag="sm", name="rstd")
                nc.vector.reciprocal(rstd[:], std[:])
                nc.vector.tensor_scalar(
                    dst, src, nm[:], rstd[:], op0=OP.add, op1=OP.mult
                )

            def pass_xtm(vsrc):
                # xT_mine = relu(v_my @ Dx)^T  [all N x my rows]
                for nt in range(NT):
                    ds = dxsp.tile([128, DU, 128], bf16, tag="dxs", name="ds")
                    nc.sync.dma_start(ds[:], dxs[nt])
                    px = psum.tile([128, R], f32, tag="mm", name="px")
                    for u in range(DU):
                        nc.tensor.matmul(
                            px[:],
                            ds[:, u],
                            vtm[:, u],
                            start=(u == 0),
                            stop=(u == DU - 1),
                        )
                    nc.scalar.activation(xtm[nt][:], px[:], AF.Relu)

            def pass_xnat(vsrc):
                # x_nat = relu(v_full @ Dx_mycols)  [all rows x my N]
                for u in range(DU):
                    nc.scalar.dma_start_transpose(
                        vtf[u][:], vsrc[:, u * 128 : (u + 1) * 128]
                    )
                for st in range(ST):
                    for nh in range(2):
                        px = psum.tile([128, 512], f32, tag="mm", name="px2")
                        for u in range(DU):
                            nc.tensor.matmul(
                                px[:],
                                vtf[u][:, st * 128 : (st + 1) * 128],
                                dxbt[:, u, nh * 512 : (nh + 1) * 512],
                                start=(u == 0),
                                stop=(u == DU - 1),
                            )
                        nc.scalar.activation(
                            xnat[st][:, nh * 512 : (nh + 1) * 512], px[:], AF.Relu
                        )

            def pass_kv(vsrc):
                # kv = x_nat^T @ v_full (local), 2 half-sweeps of 8 banks
                for h in range(2):
                    pk = [
                        psum.tile([128, 512], f32, tag="mm", name="pk")
                        for _ in range(8)
                    ]
                    for st in range(ST):
                        vn = vnp.tile([128, D], bf16, tag="vn", name="vn")
                        nc.scalar.dma_start(vn[:], vsrc[st * 128 : (st + 1) * 128, :])
                        for ntl in range(4):
                            n = h * 4 + ntl
                            for dh in range(2):
                                nc.tensor.matmul(
                                    pk[ntl * 2 + dh][:],
                                    xnat[st][:, n * 128 : (n + 1) * 128],
                                    vn[:, dh * 512 : (dh + 1) * 512],
                                    start=(st == 0),
                                    stop=(st == ST - 1),
                                )
                    for ntl in range(4):
                        n = h * 4 + ntl
                        kvt = kvtp.tile([128, D], bf16, tag="kvt", name="kvt")
                        for dh in range(2):
                            nc.vector.tensor_copy(
                                kvt[:, dh * 512 : (dh + 1) * 512], pk[ntl * 2 + dh][:]
                            )
                        nc.gpsimd.dma_start(kv_in[n // 2][n % 2], kvt[:])
                        if n % 2 == 1:
                            q = n // 2
                            nc.gpsimd.collective_compute(
                                "AllGather",
                                OP.bypass,
                                replica_groups=RG,
                                ins=[kv_in[q][:].opt()],
                                outs=[kv_out[q][:].opt()],
                            )

            def pass_a():
                # aT_mine = (x_mine @ kv)^T, 8 psum banks,
                # consuming kv chunks in sub-AG arrival order
                pa = [
                    psum.tile([128, R], f32, tag="mm", name="pa") for _ in range(DU)
                ]
                step = 0
                for q in range(4):
                    for r in range(4):
                        for j in range(2):
                            kc = kcp.tile([128, D], bf16, tag="kc", name="kc")
                            nc.scalar.dma_start(kc[:], kv_out[q][r, j])
                            nt = r * 8 + q * 2 + j
                            for dc in range(DU):
                                nc.tensor.matmul(
                                    pa[dc][:],
                                    kc[:, dc * 128 : (dc + 1) * 128],
                                    xtm[nt][:],
                                    start=(step == 0),
                                    stop=(step == 31),
                                )
                            step += 1
                for dc in range(DU):
                    nc.vector.tensor_copy(at[dc][:], pa[dc][:])

            def pass_y():
                # yT = relu(Dy^T @ aT) * xT  (in place into xtm)
                for nt in range(NT):
                    dyb = dybp.tile([128, DU, 128], bf16, tag="dyb", name="dyb")
                    nc.sync.dma_start(dyb[:], dyp[nt])
                    py = psum.tile([128, R], f32, tag="mm", name="py")
                    for u in range(DU):
                        nc.tensor.matmul(
                            py[:],
                            dyb[:, u],
                            at[u][:],
                            start=(u == 0),
                            stop=(u == DU - 1),
                        )
                    nc.scalar.activation(py[:], py[:], AF.Relu)
                    nc.vector.tensor_mul(xtm[nt][:], py[:], xtm[nt][:])

            def pass_z():
                # z = y @ E, 8 psum banks (4 rc x 2 dh)
                pz = [
                    psum.tile([128, 512], f32, tag="mm", name="pz") for _ in range(8)
                ]
                for nt in range(NT):
                    ec = ecp.tile([128, D], bf16, tag="ec", name="ec")
                    nc.sync.dma_start(ec[:], ep[nt])
                    for rc in range(RT):
                        for dh in range(2):
                            nc.tensor.matmul(
                                pz[rc * 2 + dh][:],
                                xtm[nt][:, rc * 128 : (rc + 1) * 128],
                                ec[:, dh * 512 : (dh + 1) * 512],
                                start=(nt == 0),
                                stop=(nt == NT - 1),
                            )
                return pz

            def phase4(pz, layer):
                # v = ln(v + ln(z)); bank-split first LN on PSUM
                for rc in range(RT):
                    z0, z1 = pz[rc * 2][:], pz[rc * 2 + 1][:]
                    rs0 = smpool.tile([128, 1], f32, tag="sm", name="rs0")
                    rs1 = smpool.tile([128, 1], f32, tag="sm", name="rs1")
                    nc.vector.reduce_sum(rs0[:], z0, axis=AX)
                    nc.vector.reduce_sum(rs1[:], z1, axis=AX)
                    nm = smpool.tile([128, 1], f32, tag="sm", name="nmz")
                    nc.vector.tensor_add(nm[:], rs0[:], rs1[:])
                    nc.vector.tensor_scalar_mul(nm[:], nm[:], -1.0 / D)
                    sq = lnpool.tile([128, D], f32, tag="ln", name="sqz")
                    ssq0 = smpool.tile([128, 1], f32, tag="sm", name="ssq0")
                    ssq1 = smpool.tile([128, 1], f32, tag="sm", name="ssq1")
                    nc.scalar.activation(
                        sq[:, 0:512], z0, AF.Square, bias=nm[:], scale=1.0,
                        accum_out=ssq0[:],
                    )
                    nc.scalar.activation(
                        sq[:, 512:1024], z1, AF.Square, bias=nm[:], scale=1.0,
                        accum_out=ssq1[:],
                    )
                    ssq = smpool.tile([128, 1], f32, tag="sm", name="ssqz")
                    nc.vector.tensor_add(ssq[:], ssq0[:], ssq1[:])
                    std = smpool.tile([128, 1], f32, tag="sm", name="stdz")
                    nc.scalar.activation(
                        std[:], ssq[:], AF.Sqrt, bias=epsc[:], scale=1.0 / D
                    )
                    rstd = smpool.tile([128, 1], f32, tag="sm", name="rstdz")
                    nc.vector.reciprocal(rstd[:], std[:])
                    t = lnpool.tile([128, D], f32, tag="ln", name="t")
                    nc.vector.tensor_scalar(
                        t[:, 0:512], z0, nm[:], rstd[:], op0=OP.add, op1=OP.mult
                    )
                    nc.vector.tensor_scalar(
                        t[:, 512:1024], z1, nm[:], rstd[:], op0=OP.add, op1=OP.mult
                    )
                    nc.vector.tensor_add(t[:], t[:], v_my[rc][:])
                    layer_norm(v_my[rc][:], t[:])
                    nc.vector.tensor_copy(v_bf[rc][:], v_my[rc][:])
                    if layer < layers - 1:
                        nc.gpsimd.dma_start(v_ag_in[rc], v_bf[rc][:])

                if layer < layers - 1:
                    nc.gpsimd.collective_compute(
                        "AllGather",
                        OP.bypass,
                        replica_groups=RG,
                        ins=[v_ag_in[:].opt()],
                        outs=[v_ag_out[:].opt()],
                    )

                # refresh vtm (vT of my rows) for next pass / readout
                for rc in range(RT):
                    for u in range(DU):
                        pt = psum.tile([128, 128], bf16, tag="mm", name="pt")
                        nc.tensor.transpose(
                            pt[:], v_bf[rc][:, u * 128 : (u + 1) * 128], ident[:]
                        )
                        nc.vector.tensor_copy(
                            vtm[:, u, rc * 128 : (rc + 1) * 128], pt[:]
                        )

            def readout_phase():
                # out = v @ readout
                for jv in range(VB):
                    rb = rbp.tile([128, DU, VW], bf16, tag="rb", name="rb")
                    nc.sync.dma_start(rb[:], rp[jv])
                    for rc in range(RT):
                        po = psum.tile([128, VW], f32, tag="mm", name="po")
                        for u in range(DU):
                            nc.tensor.matmul(
                                po[:],
                                vtm[:, u, rc * 128 : (rc + 1) * 128],
                                rb[:, u],
                                start=(u == 0),
                                stop=(u == DU - 1),
                            )
                        ob = opool.tile([128, VW], f32, tag="ob", name="ob")
                        nc.vector.tensor_copy(ob[:], po[:])
                        nc.sync.dma_start(out[rc, :, jv * VW : (jv + 1) * VW], ob[:])

            for layer in range(layers):
                vsrc = v0f if layer == 0 else v_ag_out
                pass_xtm(vsrc)
                pass_xnat(vsrc)
                pass_kv(vsrc)
                pass_a()
                pass_y()
                pz = pass_z()
                phase4(pz, layer)

            readout_phase()

    nc.compile()
    return nc


def get_nc(layers=L):
    if layers not in _CACHE:
        _CACHE[layers] = _build(layers=layers)
    return _CACHE[layers]


def make_in_maps(input_, emb, Dx, Dy, E, readout):
    bf = ml_dtypes.bfloat16
    idx = np.asarray(input_).astype(np.int64).reshape(-1)
    emb = np.asarray(emb, dtype=np.float32)
    v0 = emb[idx]  # [B*S, D] f32
    Dx = np.asarray(Dx, np.float32)

    dxs = np.ascontiguousarray(
        Dx.reshape(DU, 128, NT, 128).transpose(2, 1, 0, 3)
    ).astype(bf)
    dyp = np.ascontiguousarray(
        np.asarray(Dy, np.float32).reshape(DU, 128, NT, 128).transpose(2, 1, 0, 3)
    ).astype(bf)
    epp = np.ascontiguousarray(np.asarray(E, np.float32).reshape(NT, 128, D)).astype(bf)
    rpp = np.ascontiguousarray(
        np.asarray(readout, np.float32).reshape(DU, 128, VB, VW).transpose(2, 1, 0, 3)
    ).astype(bf)

    in_maps = []
    for c in range(NCORES):
        b, g = c // 4, c % 4
        vb = v0[b * S : (b + 1) * S]  # [2048, D] f32
        rows = vb[g * R : (g + 1) * R]  # [512, D]
        vmy0 = np.ascontiguousarray(rows.reshape(RT, 128, D)).astype(np.float32)
        v0tm = np.ascontiguousarray(
            rows.T.reshape(DU, 128, R).transpose(1, 0, 2)
        ).astype(bf)
        v0f = np.ascontiguousarray(vb).astype(bf)
        dxb = np.ascontiguousarray(
            Dx[:, g * 1024 : (g + 1) * 1024].reshape(DU, 128, 1024).transpose(1, 0, 2)
        ).astype(bf)
        in_maps.append(
            {
                "vmy0": vmy0,
                "v0tm": v0tm,
                "v0f": v0f,
                "dxs": dxs,
                "dxb": dxb,
                "dyp": dyp,
                "ep": epp,
                "rp": rpp,
            }
        )
    return in_maps


def kernel(input_, emb, Dx, Dy, E, readout):
    from concourse.bass_utils import run_bass_kernel_spmd

    nc = get_nc()
    in_maps = make_in_maps(input_, emb, Dx, Dy, E, readout)
    res = run_bass_kernel_spmd(nc, in_maps, core_ids=list(range(NCORES)))
    outs = [res.results[c]["out"].reshape(R, V) for c in range(NCORES)]
    return np.concatenate(outs, axis=0).reshape(B, S, V).astype(np.float32)


# revision 13
# speedup vs baseline: 1.1815x; 1.1815x over previous
"""Trainium2 Bass kernel for the 6-layer linear-attention MLP block.

Math per layer (reference):
    x  = relu(v @ Dx)                 # [S, N]
    kv = x.T @ v   (per batch)        # [N, D]   contraction over sequence
    a  = x @ kv                       # [S, D]
    y  = relu(a @ Dy) * x             # [S, N]
    v  = ln(v + ln(y @ E))            # [S, D]
final: out = v @ readout              # [S, V]

Sharding (per 4-core batch group, cores 0-3: batch 0, 4-7: batch 1):
row-shard v/a/y/z (512 rows/core) AND column-shard the kv computation
(1024 N-cols/core).  Each core computes two x pieces:
  xT_mine = x[my rows, all N]^T      (needs only my v rows)
  x_nat   = x[all rows, my N-cols]   (needs v of all rows <- AllGather(v))
kv[my cols, :] = x_nat^T @ v_full is then LOCAL in the contraction over
the full sequence -- no AllReduce.  kv chunks are AllGathered (cheap,
~87 GB/s vs AllReduce's ~32 GB/s on this fabric) in 4 pipelined
sub-AGs that overlap the a-phase, which consumes chunks in arrival
order into 8 PSUM accumulation banks.  y/z/LN are row-local; the only
other collective is AllGather(v') at the end of each layer, which rides
under the next layer's xT_mine pass (row-local, so it does not wait).

Everything computes in bf16 with f32 PSUM accumulation; the residual
stream v stays f32.  LayerNorm runs bank-split directly on the z PSUM
banks.  vT comes from HWDGE DMA-transposes of the AllGather output.
"""

import numpy as np
import ml_dtypes

B, S, N, D, V = 2, 2048, 4096, 1024, 32000
L = 6
EPS = 1e-5
NCORES = 8
R = 512    # rows per core
RT = 4     # row tiles of 128
DU = 8     # d tiles of 128
NT = 32    # global n tiles of 128
NL = 8     # local n tiles of 128 (my 1024-col slice)
ST = 16    # s tiles of 128 over the full 2048-row batch
VW = 500
VB = 64
RG = [[0, 1, 2, 3], [4, 5, 6, 7]]

_CACHE = {}


def _build(layers=L):
    import concourse.bacc as bacc
    import concourse.tile as tile
    import concourse.mybir as mybir
    from concourse.masks import make_identity

    f32 = mybir.dt.float32
    bf16 = mybir.dt.bfloat16
    AX = mybir.AxisListType.X
    AF = mybir.ActivationFunctionType
    OP = mybir.AluOpType

    nc = bacc.Bacc("TRN2", target_bir_lowering=False, num_devices=NCORES)

    vmy0 = nc.dram_tensor("vmy0", [RT, 128, D], f32, kind="ExternalInput")
    v0tm = nc.dram_tensor("v0tm", [128, DU, R], bf16, kind="ExternalInput")
    v0f = nc.dram_tensor("v0f", [S, D], bf16, kind="ExternalInput")
    dxs = nc.dram_tensor("dxs", [NT // 2, 128, 2, DU, 128], bf16, kind="ExternalInput")
    dxb = nc.dram_tensor("dxb", [128, DU, 1024], bf16, kind="ExternalInput")
    dyp = nc.dram_tensor("dyp", [NT // 2, 128, 2, DU, 128], bf16, kind="ExternalInput")
    ep = nc.dram_tensor("ep", [NT // 2, 128, 2, D], bf16, kind="ExternalInput")
    rp = nc.dram_tensor("rp", [VB, 128, DU, VW], bf16, kind="ExternalInput")
    out = nc.dram_tensor("out", [RT, 128, V], f32, kind="ExternalOutput")

    with tile.TileContext(nc) as tc:
        with (
            tc.tile_pool(name="constp", bufs=1) as constp,
            tc.tile_pool(name="pers", bufs=1) as pers,
            tc.tile_pool(name="vtfp", bufs=1) as vtfp,
            tc.tile_pool(name="xnp", bufs=1) as xnp,
            tc.tile_pool(name="xtmp", bufs=1) as xtmp,
            tc.tile_pool(name="atp", bufs=1) as atp,
            tc.tile_pool(name="dxsp", bufs=2) as dxsp,
            tc.tile_pool(name="dybp", bufs=2) as dybp,
            tc.tile_pool(name="ecp", bufs=2) as ecp,
            tc.tile_pool(name="kcp", bufs=2) as kcp,
            tc.tile_pool(name="vnp", bufs=2) as vnp,
            tc.tile_pool(name="kvtp", bufs=1) as kvtp,
            tc.tile_pool(name="rbp", bufs=2) as rbp,
            tc.tile_pool(name="opool", bufs=2) as opool,
            tc.tile_pool(name="lnpool", bufs=2) as lnpool,
            tc.tile_pool(name="smpool", bufs=12) as smpool,
            tc.tile_pool(name="psum", bufs=8, space="PSUM") as psum,
            tc.tile_pool(name="dpool", bufs=1, space="DRAM") as dpool,
        ):
            epsc = constp.tile([128, 1], f32)
            nc.vector.memset(epsc[:], EPS)
            ident = constp.tile([128, 128], bf16)
            make_identity(nc, ident)

            v_my = [pers.tile([128, D], f32, name=f"vmy{i}") for i in range(RT)]
            v_bf = [pers.tile([128, D], bf16, name=f"vbf{i}") for i in range(RT)]
            vtm = pers.tile([128, DU, R], bf16, name="vtm")
            dxbt = pers.tile([128, DU, 1024], bf16, name="dxbt")

            vtf = [vtfp.tile([128, S], bf16, name=f"vtf{i}") for i in range(DU)]
            xnat = [xnp.tile([128, 512], bf16, name=f"xn{i}") for i in range(ST)]
            xtm = [xtmp.tile([128, R], bf16, name=f"xtm{i}") for i in range(NT)]
            at = [atp.tile([128, R], bf16, name=f"at{i}") for i in range(DU)]

            kv_in = [dpool.tile([2, 128, D], bf16, name=f"kvin{q}") for q in range(4)]
            kv_out = [
                dpool.tile([4, 2, 128, D], bf16, name=f"kvout{q}") for q in range(4)
            ]
            v_ag_in = dpool.tile([RT, 128, D], bf16)
            v_ag_out = dpool.tile([S, D], bf16)

            # tiny warmup collective: absorbs the first-op trigger latency
            warm_in = dpool.tile([128, 4], f32)
            warm_out = dpool.tile([128, 4], f32)
            wt = constp.tile([128, 4], f32, name="wt")
            nc.vector.memset(wt[:], 0.0)
            nc.gpsimd.dma_start(warm_in[:], wt[:])
            nc.gpsimd.collective_compute(
                "AllReduce",
                OP.add,
                replica_groups=RG,
                ins=[warm_in[:].opt()],
                outs=[warm_out[:].opt()],
            )

            nc.sync.dma_start(dxbt[:], dxb[:])
            nc.scalar.dma_start(vtm[:], v0tm[:])
            for rc in range(RT):
                nc.gpsimd.dma_start(v_my[rc][:], vmy0[rc])

            def layer_norm(dst, src):
                rs = smpool.tile([128, 1], f32, tag="sm", name="rs")
                nc.vector.reduce_sum(rs[:], src, axis=AX)
                nm = smpool.tile([128, 1], f32, tag="sm", name="nm")
                nc.vector.tensor_scalar_mul(nm[:], rs[:], -1.0 / D)
                sq = lnpool.tile([128, D], f32, tag="ln", name="sq")
                ssq = smpool.tile([128, 1], f32, tag="sm", name="ssq")
                nc.scalar.activation(
                    sq[:], src, AF.Square, bias=nm[:], scale=1.0, accum_out=ssq[:]
                )
                std = smpool.tile([128, 1], f32, tag="sm", name="std")
                nc.scalar.activation(
                    std[:], ssq[:], AF.Sqrt, bias=epsc[:], scale=1.0 / D
                )
                rstd = smpool.tile([128, 1], f32, tag="sm", name="rstd")
                nc.vector.reciprocal(rstd[:], std[:])
                nc.vector.tensor_scalar(
                    dst, src, nm[:], rstd[:], op0=OP.add, op1=OP.mult
                )

            def pass_xtm(vsrc):
                # xT_mine = relu(v_my @ Dx)^T  [all N x my rows]
                for t in range(NT // 2):
                    ds = dxsp.tile([128, 2, DU, 128], bf16, tag="dxs", name="ds")
                    nc.sync.dma_start(ds[:], dxs[t])
                    for k in range(2):
                        nt = 2 * t + k
                        px = psum.tile([128, R], f32, tag="mm", name="px")
                        for u in range(DU):
                            nc.tensor.matmul(
                                px[:],
                                ds[:, k, u],
                                vtm[:, u],
                                start=(u == 0),
                                stop=(u == DU - 1),
                            )
                        nc.scalar.activation(xtm[nt][:], px[:], AF.Relu)

            def pass_xnat_kv(vsrc):
                # per n-half: x_nat = relu(v_full @ Dx_myhalf), then the kv
                # half-sweep consuming it (8 psum accum banks over 16 s-tiles)
                for u in range(DU):
                    nc.scalar.dma_start_transpose(
                        vtf[u][:], vsrc[:, u * 128 : (u + 1) * 128]
                    )
                for h in range(2):
                    for st in range(ST):
                        px = psum.tile([128, 512], f32, tag="mm", name="px2")
                        for u in range(DU):
                            nc.tensor.matmul(
                                px[:],
                                vtf[u][:, st * 128 : (st + 1) * 128],
                                dxbt[:, u, h * 512 : (h + 1) * 512],
                                start=(u == 0),
                                stop=(u == DU - 1),
                            )
                        nc.scalar.activation(xnat[st][:], px[:], AF.Relu)
                    pk = [
                        psum.tile([128, 512], f32, tag="mm", name="pk")
                        for _ in range(8)
                    ]
                    for t in range(ST // 2):
                        vn = vnp.tile([128, 2, D], bf16, tag="vn", name="vn")
                        nc.scalar.dma_start(
                            vn[:],
                            vsrc[t * 256 : (t + 1) * 256, :].rearrange(
                                "(k p) d -> p k d", p=128
                            ),
                        )
                        for k in range(2):
                            st = 2 * t + k
                            for ntl in range(4):
                                for dh in range(2):
                                    nc.tensor.matmul(
                                        pk[ntl * 2 + dh][:],
                                        xnat[st][:, ntl * 128 : (ntl + 1) * 128],
                                        vn[:, k, dh * 512 : (dh + 1) * 512],
                                        start=(st == 0),
                                        stop=(st == ST - 1),
                                    )
                    for ntl in range(4):
                        n = h * 4 + ntl
                        kvt = kvtp.tile([128, D], bf16, tag="kvt", name="kvt")
                        for dh in range(2):
                            nc.vector.tensor_copy(
                                kvt[:, dh * 512 : (dh + 1) * 512], pk[ntl * 2 + dh][:]
                            )
                        nc.gpsimd.dma_start(kv_in[n // 2][n % 2], kvt[:])
                        if n % 2 == 1:
                            q = n // 2
                            nc.gpsimd.collective_compute(
                                "AllGather",
                                OP.bypass,
                                replica_groups=RG,
                                ins=[kv_in[q][:].opt()],
                                outs=[kv_out[q][:].opt()],
                            )

            def pass_a():
                # aT_mine = (x_mine @ kv)^T, 8 psum banks,
                # consuming kv chunks in sub-AG arrival order
                pa = [
                    psum.tile([128, R], f32, tag="mm", name="pa") for _ in range(DU)
                ]
                step = 0
                for q in range(4):
                    for r in range(4):
                        kc = kcp.tile([128, 2, D], bf16, tag="kc", name="kc")
                        nc.scalar.dma_start(
                            kc[:], kv_out[q][r].rearrange("k p d -> p k d")
                        )
                        for j in range(2):
                            nt = r * 8 + q * 2 + j
                            for dc in range(DU):
                                nc.tensor.matmul(
                                    pa[dc][:],
                                    kc[:, j, dc * 128 : (dc + 1) * 128],
                                    xtm[nt][:],
                                    start=(step == 0),
                                    stop=(step == 31),
                                )
                            step += 1
                for dc in range(DU):
                    nc.vector.tensor_copy(at[dc][:], pa[dc][:])

            def pass_y():
                # yT = relu(Dy^T @ aT) * xT  (in place into xtm)
                for t in range(NT // 2):
                    dyb = dybp.tile([128, 2, DU, 128], bf16, tag="dyb", name="dyb")
                    nc.sync.dma_start(dyb[:], dyp[t])
                    for k in range(2):
                        nt = 2 * t + k
                        py = psum.tile([128, R], f32, tag="mm", name="py")
                        for u in range(DU):
                            nc.tensor.matmul(
                                py[:],
                                dyb[:, k, u],
                                at[u][:],
                                start=(u == 0),
                                stop=(u == DU - 1),
                            )
                        nc.scalar.activation(py[:], py[:], AF.Relu)
                        nc.vector.tensor_mul(xtm[nt][:], py[:], xtm[nt][:])

            def pass_z():
                # z = y @ E, 8 psum banks (4 rc x 2 dh)
                pz = [
                    psum.tile([128, 512], f32, tag="mm", name="pz") for _ in range(8)
                ]
                for t in range(NT // 2):
                    ec = ecp.tile([128, 2, D], bf16, tag="ec", name="ec")
                    nc.gpsimd.dma_start(ec[:], ep[t])
                    for k in range(2):
                        nt = 2 * t + k
                        for rc in range(RT):
                            for dh in range(2):
                                nc.tensor.matmul(
                                    pz[rc * 2 + dh][:],
                                    xtm[nt][:, rc * 128 : (rc + 1) * 128],
                                    ec[:, k, dh * 512 : (dh + 1) * 512],
                                    start=(nt == 0),
                                    stop=(nt == NT - 1),
                                )
                return pz

            def phase4(pz, layer):
                # v = ln(v + ln(z)); bank-split first LN on PSUM
                for rc in range(RT):
                    z0, z1 = pz[rc * 2][:], pz[rc * 2 + 1][:]
                    rs0 = smpool.tile([128, 1], f32, tag="sm", name="rs0")
                    rs1 = smpool.tile([128, 1], f32, tag="sm", name="rs1")
                    nc.vector.reduce_sum(rs0[:], z0, axis=AX)
                    nc.vector.reduce_sum(rs1[:], z1, axis=AX)
                    nm = smpool.tile([128, 1], f32, tag="sm", name="nmz")
                    nc.vector.tensor_add(nm[:], rs0[:], rs1[:])
                    nc.vector.tensor_scalar_mul(nm[:], nm[:], -1.0 / D)
                    sq = lnpool.tile([128, D], f32, tag="ln", name="sqz")
                    ssq0 = smpool.tile([128, 1], f32, tag="sm", name="ssq0")
                    ssq1 = smpool.tile([128, 1], f32, tag="sm", name="ssq1")
                    nc.scalar.activation(
                        sq[:, 0:512], z0, AF.Square, bias=nm[:], scale=1.0,
                        accum_out=ssq0[:],
                    )
                    nc.scalar.activation(
                        sq[:, 512:1024], z1, AF.Square, bias=nm[:], scale=1.0,
                        accum_out=ssq1[:],
                    )
                    ssq = smpool.tile([128, 1], f32, tag="sm", name="ssqz")
                    nc.vector.tensor_add(ssq[:], ssq0[:], ssq1[:])
                    std = smpool.tile([128, 1], f32, tag="sm", name="stdz")
                    nc.scalar.activation(
                        std[:], ssq[:], AF.Sqrt, bias=epsc[:], scale=1.0 / D
                    )
                    rstd = smpool.tile([128, 1], f32, tag="sm", name="rstdz")
                    nc.vector.reciprocal(rstd[:], std[:])
                    t = lnpool.tile([128, D], f32, tag="ln", name="t")
                    nc.vector.tensor_scalar(
                        t[:, 0:512], z0, nm[:], rstd[:], op0=OP.add, op1=OP.mult
                    )
                    nc.vector.tensor_scalar(
                        t[:, 512:1024], z1, nm[:], rstd[:], op0=OP.add, op1=OP.mult
                    )
                    nc.vector.tensor_add(t[:], t[:], v_my[rc][:])
                    layer_norm(v_my[rc][:], t[:])
                    nc.vector.tensor_copy(v_bf[rc][:], v_my[rc][:])
                    if layer < layers - 1:
                        nc.gpsimd.dma_start(v_ag_in[rc], v_bf[rc][:])

                if layer < layers - 1:
                    nc.gpsimd.collective_compute(
                        "AllGather",
                        OP.bypass,
                        replica_groups=RG,
                        ins=[v_ag_in[:].opt()],
                        outs=[v_ag_out[:].opt()],
                    )

                # refresh vtm (vT of my rows) for next pass / readout
                for rc in range(RT):
                    for u in range(DU):
                        pt = psum.tile([128, 128], bf16, tag="mm", name="pt")
                        nc.tensor.transpose(
                            pt[:], v_bf[rc][:, u * 128 : (u + 1) * 128], ident[:]
                        )
                        nc.vector.tensor_copy(
                            vtm[:, u, rc * 128 : (rc + 1) * 128], pt[:]
                        )

            def readout_phase():
                # out = v @ readout
                for jv in range(VB):
                    rb = rbp.tile([128, DU, VW], bf16, tag="rb", name="rb")
                    nc.sync.dma_start(rb[:], rp[jv])
                    for rc in range(RT):
                        po = psum.tile([128, VW], f32, tag="mm", name="po")
                        for u in range(DU):
                            nc.tensor.matmul(
                                po[:],
                                vtm[:, u, rc * 128 : (rc + 1) * 128],
                                rb[:, u],
                                start=(u == 0),
                                stop=(u == DU - 1),
                            )
                        ob = opool.tile([128, VW], f32, tag="ob", name="ob")
                        nc.vector.tensor_copy(ob[:], po[:])
                        nc.sync.dma_start(out[rc, :, jv * VW : (jv + 1) * VW], ob[:])

            for layer in range(layers):
                vsrc = v0f if layer == 0 else v_ag_out
                pass_xtm(vsrc)
                pass_xnat_kv(vsrc)
                pass_a()
                pass_y()
                pz = pass_z()
                phase4(pz, layer)

            readout_phase()

    nc.compile()
    return nc


def get_nc(layers=L):
    if layers not in _CACHE:
        _CACHE[layers] = _build(layers=layers)
    return _CACHE[layers]


def make_in_maps(input_, emb, Dx, Dy, E, readout):
    bf = ml_dtypes.bfloat16
    idx = np.asarray(input_).astype(np.int64).reshape(-1)
    emb = np.asarray(emb, dtype=np.float32)
    v0 = emb[idx]  # [B*S, D] f32
    Dx = np.asarray(Dx, np.float32)

    dxs = np.ascontiguousarray(
        Dx.reshape(DU, 128, NT // 2, 2, 128).transpose(2, 1, 3, 0, 4)
    ).astype(bf)
    dyp = np.ascontiguousarray(
        np.asarray(Dy, np.float32)
        .reshape(DU, 128, NT // 2, 2, 128)
        .transpose(2, 1, 3, 0, 4)
    ).astype(bf)
    epp = np.ascontiguousarray(
        np.asarray(E, np.float32).reshape(NT // 2, 2, 128, D).transpose(0, 2, 1, 3)
    ).astype(bf)
    rpp = np.ascontiguousarray(
        np.asarray(readout, np.float32).reshape(DU, 128, VB, VW).transpose(2, 1, 0, 3)
    ).astype(bf)

    in_maps = []
    for c in range(NCORES):
        b, g = c // 4, c % 4
        vb = v0[b * S : (b + 1) * S]  # [2048, D] f32
        rows = vb[g * R : (g + 1) * R]  # [512, D]
        vmy0 = np.ascontiguousarray(rows.reshape(RT, 128, D)).astype(np.float32)
        v0tm = np.ascontiguousarray(
            rows.T.reshape(DU, 128, R).transpose(1, 0, 2)
        ).astype(bf)
        v0f = np.ascontiguousarray(vb).astype(bf)
        dxb = np.ascontiguousarray(
            Dx[:, g * 1024 : (g + 1) * 1024].reshape(DU, 128, 1024).transpose(1, 0, 2)
        ).astype(bf)
        in_maps.append(
            {
                "vmy0": vmy0,
                "v0tm": v0tm,
                "v0f": v0f,
                "dxs": dxs,
                "dxb": dxb,
                "dyp": dyp,
                "ep": epp,
                "rp": rpp,
            }
        )
    return in_maps


def kernel(input_, emb, Dx, Dy, E, readout):
    from concourse.bass_utils import run_bass_kernel_spmd

    nc = get_nc()
    in_maps = make_in_maps(input_, emb, Dx, Dy, E, readout)
    res = run_bass_kernel_spmd(nc, in_maps, core_ids=list(range(NCORES)))
    outs = [res.results[c]["out"].reshape(R, V) for c in range(NCORES)]
    return np.concatenate(outs, axis=0).reshape(B, S, V).astype(np.float32)


# revision 14
# speedup vs baseline: 1.1938x; 1.0104x over previous
"""Trainium2 Bass kernel for the 6-layer linear-attention MLP block.

Math per layer (reference):
    x  = relu(v @ Dx)                 # [S, N]
    kv = x.T @ v   (per batch)        # [N, D]   contraction over sequence
    a  = x @ kv                       # [S, D]
    y  = relu(a @ Dy) * x             # [S, N]
    v  = ln(v + ln(y @ E))            # [S, D]
final: out = v @ readout              # [S, V]

Sharding (per 4-core batch group, cores 0-3: batch 0, 4-7: batch 1):
row-shard v/a/y/z (512 rows/core) AND column-shard the kv computation
(1024 N-cols/core).  Each core computes two x pieces:
  xT_mine = x[my rows, all N]^T      (needs only my v rows)
  x_nat   = x[all rows, my N-cols]   (needs v of all rows <- AllGather(v))
kv[my cols, :] = x_nat^T @ v_full is then LOCAL in the contraction over
the full sequence -- no AllReduce.  kv chunks are AllGathered (cheap,
~87 GB/s vs AllReduce's ~32 GB/s on this fabric) in 4 pipelined
sub-AGs that overlap the a-phase, which consumes chunks in arrival
order into 8 PSUM accumulation banks.  y/z/LN are row-local; the only
other collective is AllGather(v') at the end of each layer, which rides
under the next layer's xT_mine pass (row-local, so it does not wait).

Everything computes in bf16 with f32 PSUM accumulation; the residual
stream v stays f32.  LayerNorm runs bank-split directly on the z PSUM
banks.  vT comes from HWDGE DMA-transposes of the AllGather output.
"""

import numpy as np
import ml_dtypes

B, S, N, D, V = 2, 2048, 4096, 1024, 32000
L = 6
EPS = 1e-5
NCORES = 8
R = 512    # rows per core
RT = 4     # row tiles of 128
DU = 8     # d tiles of 128
NT = 32    # global n tiles of 128
NL = 8     # local n tiles of 128 (my 1024-col slice)
ST = 16    # s tiles of 128 over the full 2048-row batch
VW = 500
VB = 64
RG = [[0, 1, 2, 3], [4, 5, 6, 7]]

_CACHE = {}


def _build(layers=L):
    import concourse.bacc as bacc
    import concourse.tile as tile
    import concourse.mybir as mybir
    from concourse.masks import make_identity

    f32 = mybir.dt.float32
    bf16 = mybir.dt.bfloat16
    AX = mybir.AxisListType.X
    AF = mybir.ActivationFunctionType
    OP = mybir.AluOpType

    nc = bacc.Bacc("TRN2", target_bir_lowering=False, num_devices=NCORES)

    vmy0 = nc.dram_tensor("vmy0", [RT, 128, D], f32, kind="ExternalInput")
    v0tm = nc.dram_tensor("v0tm", [128, DU, R], bf16, kind="ExternalInput")
    v0f = nc.dram_tensor("v0f", [S, D], bf16, kind="ExternalInput")
    dxs = nc.dram_tensor("dxs", [NT // 4, 128, 4, DU, 128], bf16, kind="ExternalInput")
    dxb = nc.dram_tensor("dxb", [128, DU, 1024], bf16, kind="ExternalInput")
    dyp = nc.dram_tensor("dyp", [NT // 4, 128, 4, DU, 128], bf16, kind="ExternalInput")
    ep = nc.dram_tensor("ep", [NT // 2, 128, 2, D], bf16, kind="ExternalInput")
    rp = nc.dram_tensor("rp", [VB, 128, DU, VW], bf16, kind="ExternalInput")
    out = nc.dram_tensor("out", [RT, 128, V], f32, kind="ExternalOutput")

    with tile.TileContext(nc) as tc:
        with (
            tc.tile_pool(name="constp", bufs=1) as constp,
            tc.tile_pool(name="pers", bufs=1) as pers,
            tc.tile_pool(name="vtfp", bufs=3) as vtfp,
            tc.tile_pool(name="xnp", bufs=1) as xnp,
            tc.tile_pool(name="xtmp", bufs=1) as xtmp,
            tc.tile_pool(name="atp", bufs=1) as atp,
            tc.tile_pool(name="dxsp", bufs=2) as dxsp,
            tc.tile_pool(name="dybp", bufs=2) as dybp,
            tc.tile_pool(name="ecp", bufs=2) as ecp,
            tc.tile_pool(name="kcp", bufs=2) as kcp,
            tc.tile_pool(name="vnp", bufs=2) as vnp,
            tc.tile_pool(name="kvtp", bufs=1) as kvtp,
            tc.tile_pool(name="rbp", bufs=2) as rbp,
            tc.tile_pool(name="opool", bufs=2) as opool,
            tc.tile_pool(name="lnpool", bufs=2) as lnpool,
            tc.tile_pool(name="smpool", bufs=12) as smpool,
            tc.tile_pool(name="psum", bufs=8, space="PSUM") as psum,
            tc.tile_pool(name="dpool", bufs=1, space="DRAM") as dpool,
        ):
            epsc = constp.tile([128, 1], f32)
            nc.vector.memset(epsc[:], EPS)
            ident = constp.tile([128, 128], bf16)
            make_identity(nc, ident)

            v_my = [pers.tile([128, D], f32, name=f"vmy{i}") for i in range(RT)]
            v_bf = [pers.tile([128, D], bf16, name=f"vbf{i}") for i in range(RT)]
            vtm = pers.tile([128, DU, R], bf16, name="vtm")
            dxbt = pers.tile([128, DU, 1024], bf16, name="dxbt")

            xnat = [xnp.tile([128, 512], bf16, name=f"xn{i}") for i in range(ST)]
            xtm = [xtmp.tile([128, R], bf16, name=f"xtm{i}") for i in range(NT)]
            at = [atp.tile([128, R], bf16, name=f"at{i}") for i in range(DU)]

            kv_in = [dpool.tile([2, 128, D], bf16, name=f"kvin{q}") for q in range(4)]
            kv_out = [
                dpool.tile([4, 2, 128, D], bf16, name=f"kvout{q}") for q in range(4)
            ]
            v_ag_in = dpool.tile([RT, 128, D], bf16)
            v_ag_out = dpool.tile([S, D], bf16)

            # tiny warmup collective: absorbs the first-op trigger latency
            warm_in = dpool.tile([128, 4], f32)
            warm_out = dpool.tile([128, 4], f32)
            wt = constp.tile([128, 4], f32, name="wt")
            nc.vector.memset(wt[:], 0.0)
            nc.gpsimd.dma_start(warm_in[:], wt[:])
            nc.gpsimd.collective_compute(
                "AllReduce",
                OP.add,
                replica_groups=RG,
                ins=[warm_in[:].opt()],
                outs=[warm_out[:].opt()],
            )

            nc.sync.dma_start(dxbt[:], dxb[:])
            nc.scalar.dma_start(vtm[:], v0tm[:])
            for rc in range(RT):
                nc.gpsimd.dma_start(v_my[rc][:], vmy0[rc])

            def layer_norm(dst, src):
                rs = smpool.tile([128, 1], f32, tag="sm", name="rs")
                nc.vector.reduce_sum(rs[:], src, axis=AX)
                nm = smpool.tile([128, 1], f32, tag="sm", name="nm")
                nc.vector.tensor_scalar_mul(nm[:], rs[:], -1.0 / D)
                sq = lnpool.tile([128, D], f32, tag="ln", name="sq")
                ssq = smpool.tile([128, 1], f32, tag="sm", name="ssq")
                nc.scalar.activation(
                    sq[:], src, AF.Square, bias=nm[:], scale=1.0, accum_out=ssq[:]
                )
                std = smpool.tile([128, 1], f32, tag="sm", name="std")
                nc.scalar.activation(
                    std[:], ssq[:], AF.Sqrt, bias=epsc[:], scale=1.0 / D
                )
                rstd = smpool.tile([128, 1], f32, tag="sm", name="rstd")
                nc.vector.reciprocal(rstd[:], std[:])
                nc.vector.tensor_scalar(
                    dst, src, nm[:], rstd[:], op0=OP.add, op1=OP.mult
                )

            def pass_xtm(vsrc):
                # xT_mine = relu(v_my @ Dx)^T  [all N x my rows]
                for t in range(NT // 4):
                    ds = dxsp.tile([128, 4, DU, 128], bf16, tag="dxs", name="ds")
                    nc.sync.dma_start(ds[:], dxs[t])
                    for k in range(4):
                        nt = 4 * t + k
                        px = psum.tile([128, R], f32, tag="mm", name="px")
                        for u in range(DU):
                            nc.tensor.matmul(
                                px[:],
                                ds[:, k, u],
                                vtm[:, u],
                                start=(u == 0),
                                stop=(u == DU - 1),
                            )
                        nc.scalar.activation(xtm[nt][:], px[:], AF.Relu)

            def pass_xnat_kv(vsrc):
                # per n-half: x_nat = relu(v_full @ Dx_myhalf), then the kv
                # half-sweep consuming it (8 psum accum banks over 16 s-tiles)
                for h in range(2):
                    for t in range(ST // 2):
                        vt = vtfp.tile([128, DU, 256], bf16, tag="vtf", name="vt")
                        nc.scalar.dma_start_transpose(
                            vt[:], vsrc[t * 256 : (t + 1) * 256, :]
                        )
                        for k in range(2):
                            st = 2 * t + k
                            px = psum.tile([128, 512], f32, tag="mm", name="px2")
                            for u in range(DU):
                                nc.tensor.matmul(
                                    px[:],
                                    vt[:, u, k * 128 : (k + 1) * 128],
                                    dxbt[:, u, h * 512 : (h + 1) * 512],
                                    start=(u == 0),
                                    stop=(u == DU - 1),
                                )
                            nc.scalar.activation(xnat[st][:], px[:], AF.Relu)
                    pk = [
                        psum.tile([128, 512], f32, tag="mm", name="pk")
                        for _ in range(8)
                    ]
                    for t in range(ST // 2):
                        vn = vnp.tile([128, 2, D], bf16, tag="vn", name="vn")
                        nc.scalar.dma_start(
                            vn[:],
                            vsrc[t * 256 : (t + 1) * 256, :].rearrange(
                                "(k p) d -> p k d", p=128
                            ),
                        )
                        for k in range(2):
                            st = 2 * t + k
                            for ntl in range(4):
                                for dh in range(2):
                                    nc.tensor.matmul(
                                        pk[ntl * 2 + dh][:],
                                        xnat[st][:, ntl * 128 : (ntl + 1) * 128],
                                        vn[:, k, dh * 512 : (dh + 1) * 512],
                                        start=(st == 0),
                                        stop=(st == ST - 1),
                                    )
                    for ntl in range(4):
                        n = h * 4 + ntl
                        kvt = kvtp.tile([128, D], bf16, tag="kvt", name="kvt")
                        for dh in range(2):
                            nc.vector.tensor_copy(
                                kvt[:, dh * 512 : (dh + 1) * 512], pk[ntl * 2 + dh][:]
                            )
                        nc.gpsimd.dma_start(kv_in[n // 2][n % 2], kvt[:])
                        if n % 2 == 1:
                            q = n // 2
                            nc.gpsimd.collective_compute(
                                "AllGather",
                                OP.bypass,
                                replica_groups=RG,
                                ins=[kv_in[q][:].opt()],
                                outs=[kv_out[q][:].opt()],
                            )

            def pass_a():
                # aT_mine = (x_mine @ kv)^T, 8 psum banks,
                # consuming kv chunks in sub-AG arrival order
                pa = [
                    psum.tile([128, R], f32, tag="mm", name="pa") for _ in range(DU)
                ]
                step = 0
                for q in range(4):
                    for r in range(4):
                        kc = kcp.tile([128, 2, D], bf16, tag="kc", name="kc")
                        nc.scalar.dma_start(
                            kc[:], kv_out[q][r].rearrange("k p d -> p k d")
                        )
                        for j in range(2):
                            nt = r * 8 + q * 2 + j
                            for dc in range(DU):
                                nc.tensor.matmul(
                                    pa[dc][:],
                                    kc[:, j, dc * 128 : (dc + 1) * 128],
                                    xtm[nt][:],
                                    start=(step == 0),
                                    stop=(step == 31),
                                )
                            step += 1
                for dc in range(DU):
                    nc.vector.tensor_copy(at[dc][:], pa[dc][:])

            def pass_y():
                # yT = relu(Dy^T @ aT) * xT  (in place into xtm)
                for t in range(NT // 4):
                    dyb = dybp.tile([128, 4, DU, 128], bf16, tag="dyb", name="dyb")
                    nc.sync.dma_start(dyb[:], dyp[t])
                    for k in range(4):
                        nt = 4 * t + k
                        py = psum.tile([128, R], f32, tag="mm", name="py")
                        for u in range(DU):
                            nc.tensor.matmul(
                                py[:],
                                dyb[:, k, u],
                                at[u][:],
                                start=(u == 0),
                                stop=(u == DU - 1),
                            )
                        nc.scalar.activation(py[:], py[:], AF.Relu)
                        nc.vector.tensor_mul(xtm[nt][:], py[:], xtm[nt][:])

            def pass_z():
                # z = y @ E, 8 psum banks (4 rc x 2 dh)
                pz = [
                    psum.tile([128, 512], f32, tag="mm", name="pz") for _ in range(8)
                ]
                for t in range(NT // 2):
                    ec = ecp.tile([128, 2, D], bf16, tag="ec", name="ec")
                    nc.gpsimd.dma_start(ec[:], ep[t])
                    for k in range(2):
                        nt = 2 * t + k
                        for rc in range(RT):
                            for dh in range(2):
                                nc.tensor.matmul(
                                    pz[rc * 2 + dh][:],
                                    xtm[nt][:, rc * 128 : (rc + 1) * 128],
                                    ec[:, k, dh * 512 : (dh + 1) * 512],
                                    start=(nt == 0),
                                    stop=(nt == NT - 1),
                                )
                return pz

            def phase4(pz, layer):
                # v = ln(v + ln(z)); bank-split first LN on PSUM
                for rc in range(RT):
                    z0, z1 = pz[rc * 2][:], pz[rc * 2 + 1][:]
                    rs0 = smpool.tile([128, 1], f32, tag="sm", name="rs0")
                    rs1 = smpool.tile([128, 1], f32, tag="sm", name="rs1")
                    nc.vector.reduce_sum(rs0[:], z0, axis=AX)
                    nc.vector.reduce_sum(rs1[:], z1, axis=AX)
                    nm = smpool.tile([128, 1], f32, tag="sm", name="nmz")
                    nc.vector.tensor_add(nm[:], rs0[:], rs1[:])
                    nc.vector.tensor_scalar_mul(nm[:], nm[:], -1.0 / D)
                    sq = lnpool.tile([128, D], f32, tag="ln", name="sqz")
                    ssq0 = smpool.tile([128, 1], f32, tag="sm", name="ssq0")
                    ssq1 = smpool.tile([128, 1], f32, tag="sm", name="ssq1")
                    nc.scalar.activation(
                        sq[:, 0:512], z0, AF.Square, bias=nm[:], scale=1.0,
                        accum_out=ssq0[:],
                    )
                    nc.scalar.activation(
                        sq[:, 512:1024], z1, AF.Square, bias=nm[:], scale=1.0,
                        accum_out=ssq1[:],
                    )
                    ssq = smpool.tile([128, 1], f32, tag="sm", name="ssqz")
                    nc.vector.tensor_add(ssq[:], ssq0[:], ssq1[:])
                    std = smpool.tile([128, 1], f32, tag="sm", name="stdz")
                    nc.scalar.activation(
                        std[:], ssq[:], AF.Sqrt, bias=epsc[:], scale=1.0 / D
                    )
                    rstd = smpool.tile([128, 1], f32, tag="sm", name="rstdz")
                    nc.vector.reciprocal(rstd[:], std[:])
                    t = lnpool.tile([128, D], f32, tag="ln", name="t")
                    nc.vector.tensor_scalar(
                        t[:, 0:512], z0, nm[:], rstd[:], op0=OP.add, op1=OP.mult
                    )
                    nc.vector.tensor_scalar(
                        t[:, 512:1024], z1, nm[:], rstd[:], op0=OP.add, op1=OP.mult
                    )
                    nc.vector.tensor_add(t[:], t[:], v_my[rc][:])
                    layer_norm(v_my[rc][:], t[:])
                    nc.vector.tensor_copy(v_bf[rc][:], v_my[rc][:])
                    if layer < layers - 1:
                        nc.gpsimd.dma_start(v_ag_in[rc], v_bf[rc][:])

                if layer < layers - 1:
                    nc.gpsimd.collective_compute(
                        "AllGather",
                        OP.bypass,
                        replica_groups=RG,
                        ins=[v_ag_in[:].opt()],
                        outs=[v_ag_out[:].opt()],
                    )

                # refresh vtm (vT of my rows) for next pass / readout
                for rc in range(RT):
                    for u in range(DU):
                        pt = psum.tile([128, 128], bf16, tag="mm", name="pt")
                        nc.tensor.transpose(
                            pt[:], v_bf[rc][:, u * 128 : (u + 1) * 128], ident[:]
                        )
                        nc.vector.tensor_copy(
                            vtm[:, u, rc * 128 : (rc + 1) * 128], pt[:]
                        )

            def readout_phase():
                # out = v @ readout
                for jv in range(VB):
                    rb = rbp.tile([128, DU, VW], bf16, tag="rb", name="rb")
                    nc.sync.dma_start(rb[:], rp[jv])
                    for rc in range(RT):
                        po = psum.tile([128, VW], f32, tag="mm", name="po")
                        for u in range(DU):
                            nc.tensor.matmul(
                                po[:],
                                vtm[:, u, rc * 128 : (rc + 1) * 128],
                                rb[:, u],
                                start=(u == 0),
                                stop=(u == DU - 1),
                            )
                        ob = opool.tile([128, VW], f32, tag="ob", name="ob")
                        nc.vector.tensor_copy(ob[:], po[:])
                        nc.sync.dma_start(out[rc, :, jv * VW : (jv + 1) * VW], ob[:])

            for layer in range(layers):
                vsrc = v0f if layer == 0 else v_ag_out
                pass_xtm(vsrc)
                pass_xnat_kv(vsrc)
                pass_a()
                pass_y()
                pz = pass_z()
                phase4(pz, layer)

            readout_phase()

    nc.compile()
    return nc


def get_nc(layers=L):
    if layers not in _CACHE:
        _CACHE[layers] = _build(layers=layers)
    return _CACHE[layers]


def make_in_maps(input_, emb, Dx, Dy, E, readout):
    bf = ml_dtypes.bfloat16
    idx = np.asarray(input_).astype(np.int64).reshape(-1)
    emb = np.asarray(emb, dtype=np.float32)
    v0 = emb[idx]  # [B*S, D] f32
    Dx = np.asarray(Dx, np.float32)

    dxs = np.ascontiguousarray(
        Dx.reshape(DU, 128, NT // 4, 4, 128).transpose(2, 1, 3, 0, 4)
    ).astype(bf)
    dyp = np.ascontiguousarray(
        np.asarray(Dy, np.float32)
        .reshape(DU, 128, NT // 4, 4, 128)
        .transpose(2, 1, 3, 0, 4)
    ).astype(bf)
    epp = np.ascontiguousarray(
        np.asarray(E, np.float32).reshape(NT // 2, 2, 128, D).transpose(0, 2, 1, 3)
    ).astype(bf)
    rpp = np.ascontiguousarray(
        np.asarray(readout, np.float32).reshape(DU, 128, VB, VW).transpose(2, 1, 0, 3)
    ).astype(bf)

    in_maps = []
    for c in range(NCORES):
        b, g = c // 4, c % 4
        vb = v0[b * S : (b + 1) * S]  # [2048, D] f32
        rows = vb[g * R : (g + 1) * R]  # [512, D]
        vmy0 = np.ascontiguousarray(rows.reshape(RT, 128, D)).astype(np.float32)
        v0tm = np.ascontiguousarray(
            rows.T.reshape(DU, 128, R).transpose(1, 0, 2)
        ).astype(bf)
        v0f = np.ascontiguousarray(vb).astype(bf)
        dxb = np.ascontiguousarray(
            Dx[:, g * 1024 : (g + 1) * 1024].reshape(DU, 128, 1024).transpose(1, 0, 2)
        ).astype(bf)
        in_maps.append(
            {
                "vmy0": vmy0,
                "v0tm": v0tm,
                "v0f": v0f,
                "dxs": dxs,
                "dxb": dxb,
                "dyp": dyp,
                "ep": epp,
                "rp": rpp,
            }
        )
    return in_maps


def kernel(input_, emb, Dx, Dy, E, readout):
    from concourse.bass_utils import run_bass_kernel_spmd

    nc = get_nc()
    in_maps = make_in_maps(input_, emb, Dx, Dy, E, readout)
    res = run_bass_kernel_spmd(nc, in_maps, core_ids=list(range(NCORES)))
    outs = [res.results[c]["out"].reshape(R, V) for c in range(NCORES)]
    return np.concatenate(outs, axis=0).reshape(B, S, V).astype(np.float32)


# revision 15
# speedup vs baseline: 1.4802x; 1.2399x over previous
"""Trainium2 Bass kernel for the 6-layer linear-attention MLP block.

Math per layer (reference):
    x  = relu(v @ Dx)                 # [R, N]
    kv = x.T @ v   (per batch)        # [N, D]   contraction over sequence
    a  = x @ kv                       # [R, D]
    y  = relu(a @ Dy) * x             # [R, N]
    v  = ln(v + ln(y @ E))            # [R, D]
final: out = v @ readout              # [R, V]

Sharding: sequence-parallel over the 8 cores. R_global = B*S = 4096 rows;
each core owns 512 contiguous rows of one batch (cores 0-3: batch 0,
cores 4-7: batch 1). Everything is row-local except kv, which is a
partial sum over the local 512 rows -> AllReduce within each 4-core
batch group ([[0,1,2,3],[4,5,6,7]]), chunked x4 per layer for overlap.

Compute in bf16 (f32 PSUM accumulation); the residual stream v is kept
in f32. Weights are replicated and streamed from HBM each layer.
Layout transposes (x -> xT, v -> vT) run on the TensorEngine (128x128
transpose-mode matmuls); AR-gated kv loads go on the ACT HWDGE queue
and kv-partial writes on the GpSimd SWDGE queue so the in-order Sync
DMA stream never head-of-line blocks on a collective.
"""

import numpy as np
import ml_dtypes

B, S, N, D, V = 2, 2048, 4096, 1024, 32000
L = 6
EPS = 1e-5
NCORES = 8
RPC = 512  # rows per core
RT = 4     # row tiles of 128
DU = 8     # d tiles of 128
NT = 32    # n tiles of 128
NB = 8     # n blocks of 512 (4 n-tiles each)
NG = 4     # kv AllReduce chunks per layer (8 n-tiles each)
VW = 500   # vocab free-dim tile (32000 = 64*500)
VB = 64
RG = [[0, 1, 2, 3], [4, 5, 6, 7]]

_CACHE = {}


def _build(debug=False, layers=L):
    import concourse.bacc as bacc
    import concourse.tile as tile
    import concourse.mybir as mybir
    from concourse.masks import make_identity

    f32 = mybir.dt.float32
    bf16 = mybir.dt.bfloat16
    AX = mybir.AxisListType.X
    AF = mybir.ActivationFunctionType
    OP = mybir.AluOpType

    nc = bacc.Bacc("TRN2", target_bir_lowering=False, num_devices=NCORES)

    v0 = nc.dram_tensor("v0", [128, RT, D], f32, kind="ExternalInput")
    v0bf = nc.dram_tensor("v0bf", [128, RT, D], bf16, kind="ExternalInput")
    v0t = nc.dram_tensor("v0t", [RT, 128, DU, 128], bf16, kind="ExternalInput")
    dxp = nc.dram_tensor("dxp", [NB, 128, DU, 512], bf16, kind="ExternalInput")
    dyp = nc.dram_tensor("dyp", [NB, 128, DU, 512], bf16, kind="ExternalInput")
    ep = nc.dram_tensor("ep", [NT, 128, D], bf16, kind="ExternalInput")
    rp = nc.dram_tensor("rp", [VB, 128, DU, VW], bf16, kind="ExternalInput")
    out = nc.dram_tensor("out", [RT, 128, V], f32, kind="ExternalOutput")
    dbg = {}
    if debug:
        dbg["x"] = nc.dram_tensor("dbg_x", [NT, 128, RPC], bf16, kind="ExternalOutput")
        dbg["kv"] = nc.dram_tensor("dbg_kv", [N, D], bf16, kind="ExternalOutput")
        dbg["aT"] = nc.dram_tensor("dbg_aT", [DU, 128, RPC], bf16, kind="ExternalOutput")
        dbg["z"] = nc.dram_tensor("dbg_z", [RT, 128, D], f32, kind="ExternalOutput")
        dbg["v"] = nc.dram_tensor("dbg_v", [RT, 128, D], f32, kind="ExternalOutput")

    with tile.TileContext(nc) as tc:
        with (
            tc.tile_pool(name="constp", bufs=1) as constp,
            tc.tile_pool(name="pers", bufs=1) as pers,
            tc.tile_pool(name="wpool", bufs=4) as wpool,
            tc.tile_pool(name="cpool", bufs=12) as cpool,
            tc.tile_pool(name="xwpool", bufs=2) as xwpool,
            tc.tile_pool(name="ywpool", bufs=8) as ywpool,
            tc.tile_pool(name="stpool", bufs=5) as stpool,
            tc.tile_pool(name="opool", bufs=4) as opool,
            tc.tile_pool(name="lnpool", bufs=3) as lnpool,
            tc.tile_pool(name="smpool", bufs=12) as smpool,
            tc.tile_pool(name="psmm", bufs=6, space="PSUM") as psmm,
            tc.tile_pool(name="pstr", bufs=2, space="PSUM") as pstr,
            tc.tile_pool(name="dpool", bufs=1, space="DRAM") as dpool,
        ):
            epsc = constp.tile([128, 1], f32)
            nc.vector.memset(epsc[:], EPS)
            ident = constp.tile([128, 128], bf16)
            make_identity(nc, ident)

            v_f32 = [pers.tile([128, D], f32, name=f"vf{i}") for i in range(RT)]
            v_bf = [pers.tile([128, D], bf16, name=f"vb{i}") for i in range(RT)]
            vT = [pers.tile([128, DU, 128], bf16, name=f"vT{i}") for i in range(RT)]
            xT = [pers.tile([128, RPC], bf16, name=f"xT{i}") for i in range(NT)]
            aT_f = [pers.tile([128, RPC], f32, name=f"aTf{i}") for i in range(DU)]
            aT_bf = [pers.tile([128, RPC], bf16, name=f"aTb{i}") for i in range(DU)]
            z_f = [pers.tile([128, D], f32, name=f"zf{i}") for i in range(RT)]

            kv_part = dpool.tile([N, D], bf16)
            kv_red = dpool.tile([N, D], bf16)

            # tiny warmup collective: absorbs the first-op trigger latency
            warm_in = dpool.tile([128, 4], f32)
            warm_out = dpool.tile([128, 4], f32)
            wt = constp.tile([128, 4], f32, name="wt")
            nc.vector.memset(wt[:], 0.0)
            nc.gpsimd.dma_start(warm_in[:], wt[:])
            nc.gpsimd.collective_compute(
                "AllReduce",
                OP.add,
                replica_groups=RG,
                ins=[warm_in[:].opt()],
                outs=[warm_out[:].opt()],
            )

            for rt in range(RT):
                nc.sync.dma_start(vT[rt][:], v0t[rt])
            for rt in range(RT):
                nc.scalar.dma_start(v_bf[rt][:], v0bf[:, rt])
                nc.gpsimd.dma_start(v_f32[rt][:], v0[:, rt])

            def make_vT(rc):
                # vT[rc][dp, u, i] = v_bf[rc][i, u*128+dp]  (PE transpose)
                for u in range(DU):
                    pt = pstr.tile([128, 128], bf16, tag="ptr", name="pt")
                    nc.tensor.transpose(
                        pt[:], v_bf[rc][:, u * 128 : (u + 1) * 128], ident[:]
                    )
                    nc.vector.tensor_copy(vT[rc][:, u, :], pt[:])

            for layer in range(layers):
                # ---- phase 1: x = relu(v @ Dx) (per n-block), x^T, kv partial, AR
                for j in range(NB):
                    dxb = wpool.tile([128, DU, 512], bf16, tag="wblk", name="dxb")
                    nc.sync.dma_start(dxb[:], dxp[j])
                    xw = xwpool.tile([128, RT, 512], bf16, tag="xw", name="xw")
                    for rt in range(RT):
                        px = psmm.tile([128, 512], f32, tag="mm", name="px")
                        for u in range(DU):
                            nc.tensor.matmul(
                                px[:],
                                vT[rt][:, u, :],
                                dxb[:, u],
                                start=(u == 0),
                                stop=(u == DU - 1),
                            )
                        nc.scalar.activation(xw[:, rt], px[:], AF.Relu)
                    # transposes x -> xT (PE transpose)
                    for c in range(4):
                        nt = j * 4 + c
                        for rt in range(RT):
                            pt = pstr.tile([128, 128], bf16, tag="ptr", name="pt")
                            nc.tensor.transpose(
                                pt[:], xw[:, rt, c * 128 : (c + 1) * 128], ident[:]
                            )
                            nc.vector.tensor_copy(
                                xT[nt][:, rt * 128 : (rt + 1) * 128], pt[:]
                            )
                    # kv partial rows for this block
                    for c in range(4):
                        nt = j * 4 + c
                        st = stpool.tile([128, D], bf16, tag="kvst", name="st")
                        for h in range(2):
                            pk = psmm.tile([128, 512], f32, tag="mm", name="pk")
                            for rt in range(RT):
                                nc.tensor.matmul(
                                    pk[:],
                                    xw[:, rt, c * 128 : (c + 1) * 128],
                                    v_bf[rt][:, h * 512 : (h + 1) * 512],
                                    start=(rt == 0),
                                    stop=(rt == RT - 1),
                                )
                            nc.vector.tensor_copy(st[:, h * 512 : (h + 1) * 512], pk[:])
                        nc.gpsimd.dma_start(kv_part[nt * 128 : (nt + 1) * 128, :], st[:])
                    if j % 2 == 1:
                        g = j // 2
                        nc.gpsimd.collective_compute(
                            "AllReduce",
                            OP.add,
                            replica_groups=RG,
                            ins=[kv_part[g * 1024 : (g + 1) * 1024, :].opt()],
                            outs=[kv_red[g * 1024 : (g + 1) * 1024, :].opt()],
                        )

                # ---- phase 2: aT = (x @ kv)^T, accumulated over kv chunks
                for g in range(NG):
                    kvs = []
                    for q in range(8):
                        nt = g * 8 + q
                        kc = cpool.tile([128, D], bf16, tag="chunk", name="kc")
                        nc.scalar.dma_start(kc[:], kv_red[nt * 128 : (nt + 1) * 128, :])
                        kvs.append(kc)
                    for dc in range(DU):
                        pa = psmm.tile([128, 512], f32, tag="mm", name="pa")
                        for q in range(8):
                            nt = g * 8 + q
                            nc.tensor.matmul(
                                pa[:],
                                kvs[q][:, dc * 128 : (dc + 1) * 128],
                                xT[nt][:],
                                start=(q == 0),
                                stop=(q == 7),
                            )
                        if g == 0:
                            nc.vector.tensor_copy(aT_f[dc][:], pa[:])
                        else:
                            nc.vector.tensor_add(aT_f[dc][:], aT_f[dc][:], pa[:])
                        if g == NG - 1:
                            nc.vector.tensor_copy(aT_bf[dc][:], aT_f[dc][:])

                # ---- phase 3: yT = relu(Dy^T aT) * xT ; z += y @ E (grouped)
                for g in range(NG):
                    dybs = []
                    for jj in range(2):
                        dyb = wpool.tile([128, DU, 512], bf16, tag="wblk", name="dyb")
                        nc.sync.dma_start(dyb[:], dyp[2 * g + jj])
                        dybs.append(dyb)
                    yws = []
                    ecs = []
                    for q in range(8):
                        nt = g * 8 + q
                        c = nt % 4
                        dyb = dybs[(nt // 4) - 2 * g]
                        py = psmm.tile([128, 512], f32, tag="mm", name="py")
                        for u in range(DU):
                            nc.tensor.matmul(
                                py[:],
                                dyb[:, u, c * 128 : (c + 1) * 128],
                                aT_bf[u][:],
                                start=(u == 0),
                                stop=(u == DU - 1),
                            )
                        nc.scalar.activation(py[:], py[:], AF.Relu)
                        yw = ywpool.tile([128, 512], bf16, tag="yw", name="yw")
                        nc.vector.tensor_mul(yw[:], py[:], xT[nt][:])
                        yws.append(yw)
                        ec = cpool.tile([128, D], bf16, tag="chunk", name="ec")
                        nc.sync.dma_start(ec[:], ep[nt])
                        ecs.append(ec)
                    for rc in range(RT):
                        for h in range(2):
                            pz = psmm.tile([128, 512], f32, tag="mm", name="pz")
                            for q in range(8):
                                nc.tensor.matmul(
                                    pz[:],
                                    yws[q][:, rc * 128 : (rc + 1) * 128],
                                    ecs[q][:, h * 512 : (h + 1) * 512],
                                    start=(q == 0),
                                    stop=(q == 7),
                                )
                            zs = z_f[rc][:, h * 512 : (h + 1) * 512]
                            if g == 0:
                                nc.vector.tensor_copy(zs, pz[:])
                            else:
                                nc.vector.tensor_add(zs, zs, pz[:])

                if debug and layer == 0:
                    for nt in range(NT):
                        nc.sync.dma_start(dbg["x"][nt], xT[nt][:])
                    nc.sync.dma_start(dbg["kv"][:], kv_red[:])
                    for dc in range(DU):
                        nc.sync.dma_start(dbg["aT"][dc], aT_bf[dc][:])
                    for rc in range(RT):
                        nc.sync.dma_start(dbg["z"][rc], z_f[rc][:])

                # ---- phase 4: v = ln(v + ln(z)) rowwise; then refresh v_bf/vT
                def layer_norm(dst, src):
                    rs = smpool.tile([128, 1], f32, tag="sm", name="rs")
                    nc.vector.reduce_sum(rs[:], src, axis=AX)
                    nm = smpool.tile([128, 1], f32, tag="sm", name="nm")
                    nc.vector.tensor_scalar_mul(nm[:], rs[:], -1.0 / D)
                    sq = lnpool.tile([128, D], f32, tag="ln", name="sq")
                    ssq = smpool.tile([128, 1], f32, tag="sm", name="ssq")
                    nc.scalar.activation(
                        sq[:], src, AF.Square, bias=nm[:], scale=1.0, accum_out=ssq[:]
                    )
                    std = smpool.tile([128, 1], f32, tag="sm", name="std")
                    nc.scalar.activation(
                        std[:], ssq[:], AF.Sqrt, bias=epsc[:], scale=1.0 / D
                    )
                    rstd = smpool.tile([128, 1], f32, tag="sm", name="rstd")
                    nc.vector.reciprocal(rstd[:], std[:])
                    nc.vector.tensor_scalar(
                        dst, src, nm[:], rstd[:], op0=OP.add, op1=OP.mult
                    )

                for rc in range(RT):
                    t = lnpool.tile([128, D], f32, tag="ln", name="t")
                    layer_norm(t[:], z_f[rc][:])
                    nc.vector.tensor_add(t[:], t[:], v_f32[rc][:])
                    layer_norm(v_f32[rc][:], t[:])
                    nc.vector.tensor_copy(v_bf[rc][:], v_f32[rc][:])
                    make_vT(rc)

            if debug:
                for rc in range(RT):
                    nc.sync.dma_start(dbg["v"][rc], v_f32[rc][:])

            # ---- readout: out = v @ readout
            for jv in range(VB):
                rb = wpool.tile([128, DU, VW], bf16, tag="wblk", name="rb")
                nc.gpsimd.dma_start(rb[:], rp[jv])
                for rc in range(RT):
                    po = psmm.tile([128, VW], f32, tag="mm", name="po")
                    for u in range(DU):
                        nc.tensor.matmul(
                            po[:],
                            vT[rc][:, u, :],
                            rb[:, u],
                            start=(u == 0),
                            stop=(u == DU - 1),
                        )
                    ob = opool.tile([128, VW], f32, tag="ob", name="ob")
                    nc.vector.tensor_copy(ob[:], po[:])
                    nc.sync.dma_start(out[rc, :, jv * VW : (jv + 1) * VW], ob[:])

    nc.compile()
    return nc


def get_nc(debug=False, layers=L):
    key = (debug, layers)
    if key not in _CACHE:
        _CACHE[key] = _build(debug=debug, layers=layers)
    return _CACHE[key]


def make_in_maps(input_, emb, Dx, Dy, E, readout):
    bf = ml_dtypes.bfloat16
    idx = np.asarray(input_).astype(np.int64).reshape(-1)
    emb = np.asarray(emb, dtype=np.float32)
    v0 = emb[idx]  # [B*S, D] f32

    dxp = np.ascontiguousarray(
        np.asarray(Dx, np.float32).reshape(DU, 128, NB, 512).transpose(2, 1, 0, 3)
    ).astype(bf)
    dyp = np.ascontiguousarray(
        np.asarray(Dy, np.float32).reshape(DU, 128, NB, 512).transpose(2, 1, 0, 3)
    ).astype(bf)
    epp = np.ascontiguousarray(np.asarray(E, np.float32).reshape(NT, 128, D)).astype(bf)
    rpp = np.ascontiguousarray(
        np.asarray(readout, np.float32).reshape(DU, 128, VB, VW).transpose(2, 1, 0, 3)
    ).astype(bf)

    in_maps = []
    for c in range(NCORES):
        rows = v0[c * RPC : (c + 1) * RPC]  # [512, D] f32
        v0p = np.ascontiguousarray(
            rows.reshape(RT, 128, D).transpose(1, 0, 2)
        ).astype(np.float32)
        v0pbf = v0p.astype(bf)
        # v0t[rt][p, u, i] = rows[rt*128+i, u*128+p]
        v0t = np.ascontiguousarray(
            rows.reshape(RT, 128, DU, 128).transpose(0, 3, 2, 1)
        ).astype(bf)
        in_maps.append(
            {"v0": v0p, "v0bf": v0pbf, "v0t": v0t,
             "dxp": dxp, "dyp": dyp, "ep": epp, "rp": rpp}
        )
    return in_maps


def kernel(input_, emb, Dx, Dy, E, readout):
    from concourse.bass_utils import run_bass_kernel_spmd

    nc = get_nc()
    in_maps = make_in_maps(input_, emb, Dx, Dy, E, readout)
    res = run_bass_kernel_spmd(nc, in_maps, core_ids=list(range(NCORES)))
    outs = [res.results[c]["out"].reshape(RPC, V) for c in range(NCORES)]
    return np.concatenate(outs, axis=0).reshape(B, S, V).astype(np.float32)

